# revision 17
# baseline (speedup 1.0000x reference)
"""Trainium2 Bass kernel for nn_Attention (dual-softmax linear attention).

Reference computation (per batch b):
  q  = x @ Wq                    [S, DM]   (DM = H*DH = 1024)
  kv = x @ Wkv                   [S, 2*DM] -> per head h: cols [h*128, h*128+64) = k_h,
                                              cols [h*128+64, (h+1)*128) = v_h
  q  = softmax(q over dh) * DH^-0.5
  k  = softmax(k over s)
  ctx_h   = k_h^T @ v_h          [DH, DH]
  out_h   = q_h @ ctx_h          [S, DH]
  y  = out @ Wlin + blin         [S, DM]

Sharding: data-parallel over batch B=8 -> one batch element per NeuronCore.

All three big projections (kv, q, y) run as fp8e4 DoubleRow matmuls
(2 fp8 weights per PE cell, 256-deep contraction per pass, ~2x the
fp16 issue rate). fp8 precision is preserved via an exact common-mode
decomposition computed host-side in fp32:

  * x is centered per-column: xc = x - mean_s(x). The k-softmax over s
    is shift-invariant, so exp(k') needs no correction. The q logits
    get their exact mean back via the ScalarE Exp bias (per-partition
    qbar = mean_s(x) @ Wq). v' = xc @ Wv are deviations; the common row
    vbar = mean_s(x) @ Wv is carried exactly through the host-computed
    output bias row (w2bar_sum = SCALE * vbar @ Wlin per head, summed).
  * W2 = (ctx' @ Wlin) * SCALE/colsum keeps only the deviation part on
    device (fp8, scale 2^18); the common part never exists on device.
  * y = eqn8 @ W2d (fp8 DoubleRow, scale 2^26) is descaled on ScalarE
    and biased with ybias = w2bar_sum + blin (exact fp32, broadcast).

Host also pre-transposes + pair-interleaves x into the DoubleRow layout
(logical contraction row d = jp*256 + ko*128 + p), removing all device
transposes of x. Simulated end-to-end rel err: 3.2e-4 (tolerance 2e-2).

ctx'/colsum share one stationary (ek) per (j, t): ctx' lands [d, e]
plus a colsum column; ctx_acc is transposed once per j on the PE (fp32
transpose mode) before the W2 matmul. Other small matmuls (rowsum, W2)
are fp16 with fp32 PSUM.

DMA: x chunk tiles ride the ScalarE HWDGE queue, weights + y the SP
queue, bias broadcasts the GpSimd queue; all tiles are laid out
per-partition-contiguous in DRAM for full-size descriptors.
"""

import numpy as np
import ml_dtypes

import concourse.bass as bass
import concourse.mybir as mybir
from concourse import bacc
from concourse.tile import TileContext
from concourse.masks import make_identity

F32 = mybir.dt.float32
F16 = mybir.dt.float16
F8 = mybir.dt.float8e4
AF = mybir.ActivationFunctionType
DR = mybir.MatmulPerfMode.DoubleRow

S, D = 4096, 1024
H, DH = 16, 64
DM = H * DH  # 1024
B = 8
SCALE = DH ** (-0.5)

P = 128          # partitions
NB = 512         # moving free-dim tile
NJP = 4          # d-tile pairs (DoubleRow contracts 256 at a time)
NJ = DM // P     # 8 dout-tiles (head pairs)
HH = H // 2      # heads per kv half-tile

# fp8 scales (power-of-2, validated by numeric sim against this input
# distribution; inputs clipped to +-240 on conversion)
SX = 2.0 ** 7        # x (centered) -> fp8
SW = 2.0 ** 10       # Wkv / Wq -> fp8
KV_INV = 2.0 ** -17  # descale for kv / q PSUM (1 / (SX * SW))
SEQ = 2.0 ** 8       # eqn -> fp8 (folded into blkones = 2^-8)
SW2 = 2.0 ** 18      # W2 deviations -> fp8
Y_INV = 2.0 ** -26   # descale for y PSUM (1 / (SEQ * SW2))


def build_nc(s_len=S):
    sc = s_len // NB
    nc = bacc.Bacc(None, target_bir_lowering=False)

    x_in = nc.declare_dram_parameter("x", [sc, P, NJP, 2, NB], F8, isOutput=False)
    wq_in = nc.declare_dram_parameter("Wq", [NJP, P, 2, DM], F8, isOutput=False)
    wkv_in = nc.declare_dram_parameter("Wkv", [2, NJP, P, 2, DM], F8, isOutput=False)
    wlin_in = nc.declare_dram_parameter("Wlin", [DM, DM], F16, isOutput=False)
    qbar_in = nc.declare_dram_parameter("qbar", [NJ, P], F32, isOutput=False)
    ybias_in = nc.declare_dram_parameter("ybias", [1, DM], F32, isOutput=False)
    # tile-major output layout: [c, p, t, m] so each partition's DMA run is
    # 8KB contiguous (host inverse-permutes to [s, m] for free)
    y_out = nc.declare_dram_parameter("y", [sc, P, 4 * DM], F16, isOutput=True)

    with TileContext(nc) as tc:
        from contextlib import ExitStack

        with ExitStack() as stk:
            consts = stk.enter_context(tc.tile_pool(name="consts", bufs=1))
            wbig = stk.enter_context(tc.tile_pool(name="wbig", bufs=1))
            wqp = stk.enter_context(tc.tile_pool(name="wqp", bufs=1))
            wlp = stk.enter_context(tc.tile_pool(name="wlp", bufs=1))

            # blkones: per-head rowsum blocks scaled by 2^-8 so the
            # reciprocal yields 2^8/rowsum (folds the eqn fp8 scale)
            blkones = consts.tile([P, P], F16, tag="blkones")
            nc.vector.memset(blkones, 0.0)
            nc.vector.memset(blkones[0:64, 0:64], 1.0 / SEQ)
            nc.vector.memset(blkones[64:128, 64:128], 1.0 / SEQ)
            onescol = consts.tile([P, 1], F16, tag="onescol")
            nc.vector.memset(onescol, 1.0)
            ident32 = consts.tile([P, P], F32, tag="ident32")
            make_identity(nc, ident32)

            # ybias broadcast to all partitions via step-0 partition DMA
            ybias_bc = consts.tile([P, DM], F32, tag="ybias_bc")
            ybias_row = ybias_in[0, :]
            ybias_bcast_ap = bass.AP(
                tensor=ybias_row.tensor,
                offset=ybias_row.offset,
                ap=[[0, P]] + list(ybias_row.ap),
            )
            nc.gpsimd.dma_start(out=ybias_bc, in_=ybias_bcast_ap)

            # per-partition Exp bias columns (qbar), one col per j-group
            qbar_sb = consts.tile([P, NJ], F32, tag="qbar_sb")
            for j in range(NJ):
                nc.gpsimd.dma_start(out=qbar_sb[:, j:j + 1], in_=qbar_in[j, :])

            # blockdiag ctx staging tiles (off-diagonal zeros preset)
            bdt_tiles = []
            for j in range(NJ):
                bdt = consts.tile([P, P], F16, tag=f"bdt{j}", name=f"bdt{j}")
                nc.vector.memset(bdt, 0.0)
                bdt_tiles.append(bdt)

            # ctx [d, e] / colsum [d] accumulators (SBUF, fp32)
            ctx_acc = []
            cs_acc = []
            for j in range(NJ):
                ca = consts.tile([P, P], F32, tag=f"ctx_acc{j}")
                nc.vector.memset(ca, 0.0)
                ctx_acc.append(ca)
                cs = consts.tile([P, 1], F32, tag=f"cs_acc{j}")
                nc.vector.memset(cs, 0.0)
                cs_acc.append(cs)
            rcs2 = consts.tile([P, NJ], F32, tag="rcs2")

            # weights resident for phase A (fp8 DoubleRow pair layout;
            # wkv is h2-major so each DMA is partition-contiguous)
            wkv_sb = [
                wbig.tile([P, 2, 2, DM], F8, tag=f"w{jp}", name=f"wkv{jp}")
                for jp in range(NJP)
            ]
            wq_sb = [
                wqp.tile([P, 2, DM], F8, tag=f"q{jp}", name=f"wq{jp}")
                for jp in range(NJP)
            ]
            wlin_sb = [
                wlp.tile([P, DM], F16, tag=f"l{j}", name=f"wlin{j}")
                for j in range(NJ)
            ]

            def load_wkv():
                for h2 in range(2):
                    for jp in range(NJP):
                        nc.sync.dma_start(
                            out=wkv_sb[jp][:, h2, :, :],
                            in_=wkv_in[h2, jp],
                        )

            def load_wq():
                for jp in range(NJP):
                    nc.sync.dma_start(out=wq_sb[jp], in_=wq_in[jp])

            xt_pool = stk.enter_context(tc.tile_pool(name="xt", bufs=3))
            ek_pool = stk.enter_context(tc.tile_pool(name="ek", bufs=1))
            vt_pool = stk.enter_context(tc.tile_pool(name="vt", bufs=1))
            eq16_pool = stk.enter_context(tc.tile_pool(name="eq16", bufs=3))
            eqres_pool = stk.enter_context(tc.tile_pool(name="eqres", bufs=1))
            rr_pool = stk.enter_context(tc.tile_pool(name="rr", bufs=2))
            eq_res = [[None] * NJP for _ in range(sc)]

            w2_sb = [
                consts.tile([P, 2, DM], F8, tag=f"w2_{jp}", name=f"w2_{jp}")
                for jp in range(NJP)
            ]

            # ---------------- phase A ----------------
            with (
                tc.tile_pool(name="kvp", bufs=2, space="PSUM") as kvp_pool,
                tc.tile_pool(name="ctxp", bufs=2, space="PSUM") as ctxp_pool,
                tc.tile_pool(name="qp", bufs=2, space="PSUM") as qp_pool,
            ):
                # x chunk tiles: one 4KB-per-partition DMA per chunk on the
                # ACT HWDGE queue, prefetched two chunks ahead
                xt_bufs = [None] * sc

                def load_xt(c):
                    if c >= sc:
                        return
                    xt_b = xt_pool.tile([P, NJP, 2, NB], F8, tag="xt")
                    nc.scalar.dma_start(out=xt_b, in_=x_in[c])
                    xt_bufs[c] = xt_b

                load_xt(0)
                load_xt(1)
                for c in range(sc):
                    load_xt(c + 2)
                    xt_big = xt_bufs[c]
                    xt_tiles = [xt_big[:, jp] for jp in range(NJP)]
                    if c == 0:
                        load_wkv()
                        load_wq()
                    if c == sc - 1:
                        for j in range(NJ):
                            nc.sync.dma_start(
                                out=wlin_sb[j], in_=wlin_in[j * P:(j + 1) * P, :]
                            )

                    # kv projection (fp8 DoubleRow), two 1024-wide halves;
                    # h2-major so chunk 0 starts on the first half of Wkv
                    # while the second half is still streaming in
                    ek_tiles = [[None, None] for _ in range(4)]
                    v_tiles = [[None, None] for _ in range(4)]
                    for h2 in range(2):
                        for t in range(4):
                            kvps = kvp_pool.tile([P, DM], F32, tag="kvp")
                            for n in range(2):
                                for jp in range(NJP):
                                    nc.tensor.matmul(
                                        kvps[:, n * NB:(n + 1) * NB],
                                        xt_tiles[jp][:, :, t * P:(t + 1) * P],
                                        wkv_sb[jp][:, h2, :, n * NB:(n + 1) * NB],
                                        start=(jp == 0),
                                        stop=(jp == NJP - 1),
                                        perf_mode=DR,
                                    )
                            kv3 = kvps.rearrange("p (h c) -> p h c", h=HH)
                            ek_t = ek_pool.tile([P, HH, DH], F16, tag=f"ek{t}_{h2}")
                            nc.scalar.activation(
                                ek_t, kv3[:, :, 0:DH], AF.Exp, scale=KV_INV
                            )
                            v_t = vt_pool.tile([P, HH, DH], F16, tag=f"v{t}_{h2}")
                            nc.scalar.activation(
                                v_t, kv3[:, :, DH:2 * DH], AF.Copy, scale=KV_INV
                            )
                            ek_tiles[t][h2] = ek_t.rearrange("p h c -> p (h c)")
                            v_tiles[t][h2] = v_t.rearrange("p h c -> p (h c)")

                    # ctx^T [d, e] + colsum [d] share the ek stationary
                    for j in range(NJ):
                        h2, jl = j // 4, j % 4
                        cps = ctxp_pool.tile([P, P + 4], F32, tag="ctxp")
                        for t in range(4):
                            st = ek_tiles[t][h2][:, jl * P:(jl + 1) * P]
                            nc.tensor.matmul(
                                cps[:, 0:P],
                                st,
                                v_tiles[t][h2][:, jl * P:(jl + 1) * P],
                                start=(t == 0),
                                stop=False,
                            )
                            # start=False: ctx's t==0 start already cleared
                            # this bank's has_written bits, so the first
                            # colsum write overwrites rather than accumulates
                            nc.tensor.matmul(
                                cps[:, P:P + 1],
                                st,
                                onescol,
                                start=False,
                                stop=(t == 3),
                            )
                        nc.vector.tensor_add(ctx_acc[j], ctx_acc[j], cps[:, 0:P])
                        nc.vector.tensor_add(cs_acc[j], cs_acc[j], cps[:, P:P + 1])

                    if c == sc - 1:
                        # start the W2 finalize critical path early: colsum
                        # reciprocals (DVE) + ctx transposes (PE, fp32 mode)
                        # overlap the last chunk's q projection
                        for j in range(NJ):
                            rcs = rr_pool.tile([P, 1], F32, tag="rcs")
                            nc.vector.reciprocal_approx_fast(
                                out=rcs, in_=cs_acc[j]
                            )
                            nc.vector.tensor_scalar(
                                out=rcs2[:, j:j + 1],
                                in0=rcs,
                                scalar1=SCALE * SW2,
                                scalar2=None,
                                op0=mybir.AluOpType.mult,
                            )
                        for j in range(NJ):
                            tp = ctxp_pool.tile([P, P + 4], F32, tag="ctxp")
                            nc.tensor.transpose(
                                tp[:, 0:P], ctx_acc[j], ident32
                            )
                            nc.vector.tensor_copy(
                                bdt_tiles[j][0:64, 0:64], tp[0:64, 0:64]
                            )
                            nc.vector.tensor_copy(
                                bdt_tiles[j][64:128, 64:128], tp[64:128, 64:128]
                            )

                    # q projection (fp8 DoubleRow) -> eqn8 pairs; the
                    # rowsum matmul trails one j behind so the PE never
                    # waits on the ScalarE Exp of the current j
                    eq16s = [None] * NJ
                    def rowsum(j):
                        rsps = qp_pool.tile([P, NB], F32, tag="qp")
                        nc.tensor.matmul(rsps, blkones, eq16s[j])
                        rr = rr_pool.tile([P, NB], F32, tag="rr")
                        nc.vector.reciprocal_approx_fast(out=rr, in_=rsps)
                        nc.vector.tensor_mul(
                            eq_res[c][j // 2][:, j % 2, :], eq16s[j], rr
                        )
                    for j in range(NJ):
                        if j % 2 == 0:
                            eqp = eqres_pool.tile(
                                [P, 2, NB], F8, tag=f"eq{c}_{j // 2}"
                            )
                            eq_res[c][j // 2] = eqp
                        qps = qp_pool.tile([P, NB], F32, tag="qp")
                        for jp in range(NJP):
                            nc.tensor.matmul(
                                qps,
                                wq_sb[jp][:, :, j * P:(j + 1) * P],
                                xt_tiles[jp],
                                start=(jp == 0),
                                stop=(jp == NJP - 1),
                                perf_mode=DR,
                            )
                        eq16 = eq16_pool.tile([P, NB], F16, tag="eq16")
                        nc.scalar.activation(
                            eq16, qps, AF.Exp,
                            scale=KV_INV, bias=qbar_sb[:, j:j + 1],
                        )
                        eq16s[j] = eq16
                        if j > 0:
                            rowsum(j - 1)
                    rowsum(NJ - 1)

            # -------- finalize: W2 deviations -> fp8 pairs (ScalarE
            # evac with per-partition 2^18*SCALE/colsum scale) --------
            with tc.tile_pool(name="w2p", bufs=2, space="PSUM") as w2p_pool:
                for j in range(NJ):
                    w2ps = w2p_pool.tile([P, DM], F32, tag="w2p")
                    for n in range(2):
                        nc.tensor.matmul(
                            w2ps[:, n * NB:(n + 1) * NB],
                            bdt_tiles[j],
                            wlin_sb[j][:, n * NB:(n + 1) * NB],
                        )
                    # evacs split across ScalarE + DVE so phase B's first
                    # matmuls aren't gated on one engine's serial chain
                    dst = w2_sb[j // 2][:, j % 2, :]
                    if j % 2 == 0:
                        nc.scalar.activation(
                            dst, w2ps, AF.Copy, scale=rcs2[:, j:j + 1]
                        )
                    else:
                        nc.vector.tensor_scalar(
                            out=dst, in0=w2ps, scalar1=rcs2[:, j:j + 1],
                            scalar2=None, op0=mybir.AluOpType.mult,
                        )

            y_pool = stk.enter_context(tc.tile_pool(name="ysb", bufs=3))
            yd_pool = stk.enter_context(tc.tile_pool(name="yd", bufs=3))

            # ---------------- phase B: y = eqn8 @ W2d (fp8 DoubleRow) ---
            with tc.tile_pool(name="yp", bufs=3, space="PSUM") as yp_pool:
                for c in range(sc):
                    ysb4 = y_pool.tile([P, 4, DM], F16, tag="ysb")
                    for t in range(4):
                        yps = yp_pool.tile([P, DM], F32, tag="yp")
                        for n in range(2):
                            for jp in range(NJP):
                                nc.tensor.matmul(
                                    yps[:, n * NB:(n + 1) * NB],
                                    eq_res[c][jp][:, :, t * P:(t + 1) * P],
                                    w2_sb[jp][:, :, n * NB:(n + 1) * NB],
                                    start=(jp == 0),
                                    stop=(jp == NJP - 1),
                                    perf_mode=DR,
                                )
                        yd = yd_pool.tile([P, DM], F32, tag="yd")
                        nc.scalar.activation(yd, yps, AF.Copy, scale=Y_INV)
                        nc.vector.tensor_add(ysb4[:, t, :], yd, ybias_bc)
                    # one batched 8KB-per-partition DMA per chunk
                    nc.sync.dma_start(out=y_out[c], in_=ysb4)
    nc.compile()
    return nc


def _q8(a, scale):
    return np.clip(
        np.asarray(a, dtype=np.float32) * scale, -240.0, 240.0
    ).astype(ml_dtypes.float8_e4m3)


def prepare_inputs(x, Wq, Wkv, Wlin, blin):
    """Host-side packing: returns per-core input maps."""
    x = np.asarray(x, dtype=np.float32)
    s_len = x.shape[1]
    sc = s_len // NB
    Wq64 = np.asarray(Wq, dtype=np.float64)
    Wkv64 = np.asarray(Wkv, dtype=np.float64)
    Wlin64 = np.asarray(Wlin, dtype=np.float64)
    blin64 = np.asarray(blin, dtype=np.float64).reshape(DM)

    # DoubleRow pair layouts (logical contraction row d = jp*256 + ko*128 + p)
    # wkv: [h2, jp, p, ko, n] — partition-contiguous per (h2, jp) tile
    wkv8 = _q8(
        np.asarray(Wkv, np.float32).reshape(NJP, 2, P, 2, DM)
        .transpose(3, 0, 2, 1, 4), SW,
    )
    wq8 = _q8(
        np.asarray(Wq, np.float32).reshape(NJP, 2, P, DM)
        .transpose(0, 2, 1, 3), SW,
    )
    wlin16 = np.asarray(Wlin, np.float32).astype(np.float16)

    in_maps = []
    for b in range(x.shape[0]):
        xb = x[b].astype(np.float64)
        xm = xb.mean(axis=0)                      # [D]
        xc = (xb - xm).astype(np.float32)
        # x: [c, p, jp, ko, s'] — one 4KB-per-partition tile per chunk
        xdr = _q8(
            np.ascontiguousarray(
                xc.T.reshape(NJP, 2, P, sc, NB).transpose(3, 2, 0, 1, 4)
            ), SX,
        )
        qbar = (xm @ Wq64).astype(np.float32).reshape(NJ, P)
        vbar = (xm @ Wkv64).reshape(H, 2 * DH)[:, DH:]        # [H, DH]
        w2bar_sum = SCALE * (vbar.reshape(DM) @ Wlin64)       # [DM]
        ybias = (w2bar_sum + blin64).astype(np.float32).reshape(1, DM)
        in_maps.append({
            "x": np.ascontiguousarray(xdr),
            "Wq": np.ascontiguousarray(wq8),
            "Wkv": np.ascontiguousarray(wkv8),
            "Wlin": wlin16,
            "qbar": np.ascontiguousarray(qbar),
            "ybias": ybias,
        })
    return in_maps


def kernel(x, Wq, Wkv, Wlin, blin):
    from concourse.bass_utils import run_bass_kernel_spmd

    x = np.asarray(x, dtype=np.float32)
    b = x.shape[0]
    nc = build_nc(x.shape[1])
    in_maps = prepare_inputs(x, Wq, Wkv, Wlin, blin)
    res = run_bass_kernel_spmd(nc, in_maps, list(range(b)))
    sc = x.shape[1] // NB
    return np.stack([
        res.results[i]["y"].astype(np.float32)
        .reshape(sc, P, 4, DM).transpose(0, 2, 1, 3).reshape(x.shape[1], DM)
        for i in range(b)
    ])


if __name__ == "__main__":
    rng = np.random.default_rng(0)
    x = rng.random((B, S, D), dtype=np.float32)
    Wq = (rng.standard_normal((D, DM)) * 0.02).astype(np.float32)
    Wkv = (rng.standard_normal((D, 2 * DM)) * 0.02).astype(np.float32)
    Wlin = (rng.standard_normal((DM, DM)) * 0.02).astype(np.float32)
    blin = np.zeros((DM,), dtype=np.float32)
    y = kernel(x=x, Wq=Wq, Wkv=Wkv, Wlin=Wlin, blin=blin)
    print(y.shape, y.dtype)


# revision 19
# speedup vs baseline: 1.0103x; 1.0103x over previous
"""Trainium2 Bass kernel for nn_Attention (dual-softmax linear attention).

Reference computation (per batch b):
  q  = x @ Wq                    [S, DM]   (DM = H*DH = 1024)
  kv = x @ Wkv                   [S, 2*DM] -> per head h: cols [h*128, h*128+64) = k_h,
                                              cols [h*128+64, (h+1)*128) = v_h
  q  = softmax(q over dh) * DH^-0.5
  k  = softmax(k over s)
  ctx_h   = k_h^T @ v_h          [DH, DH]
  out_h   = q_h @ ctx_h          [S, DH]
  y  = out @ Wlin + blin         [S, DM]

Sharding: data-parallel over batch B=8 -> one batch element per NeuronCore.

All three big projections (kv, q, y) run as fp8e4 DoubleRow matmuls
(2 fp8 weights per PE cell, 256-deep contraction per pass, ~2x the
fp16 issue rate). fp8 precision is preserved via an exact common-mode
decomposition computed host-side in fp32:

  * x is centered per-column: xc = x - mean_s(x). The k-softmax over s
    is shift-invariant, so exp(k') needs no correction. The q logits
    get their exact mean back via the ScalarE Exp bias (per-partition
    qbar = mean_s(x) @ Wq). v' = xc @ Wv are deviations; the common row
    vbar = mean_s(x) @ Wv is carried exactly through the host-computed
    output bias row (w2bar_sum = SCALE * vbar @ Wlin per head, summed).
  * W2 = (ctx' @ Wlin) * SCALE/colsum keeps only the deviation part on
    device (fp8, scale 2^18); the common part never exists on device.
  * y = eqn8 @ W2d (fp8 DoubleRow, scale 2^26) is descaled on ScalarE
    and biased with ybias = w2bar_sum + blin (exact fp32, broadcast).

Host also pre-transposes + pair-interleaves x into the DoubleRow layout
(logical contraction row d = jp*256 + ko*128 + p), removing all device
transposes of x. Simulated end-to-end rel err: 3.2e-4 (tolerance 2e-2).

ctx'/colsum share one stationary (ek) per (j, t): ctx' lands [d, e]
plus a colsum column; ctx_acc is transposed once per j on the PE (fp32
transpose mode) before the W2 matmul. Other small matmuls (rowsum, W2)
are fp16 with fp32 PSUM.

DMA: x chunk tiles ride the ScalarE HWDGE queue, weights + y the SP
queue, bias broadcasts the GpSimd queue; all tiles are laid out
per-partition-contiguous in DRAM for full-size descriptors.
"""

import numpy as np
import ml_dtypes

import concourse.bass as bass
import concourse.mybir as mybir
from concourse import bacc
from concourse.tile import TileContext
from concourse.masks import make_identity

F32 = mybir.dt.float32
F16 = mybir.dt.float16
F8 = mybir.dt.float8e4
AF = mybir.ActivationFunctionType
DR = mybir.MatmulPerfMode.DoubleRow

S, D = 4096, 1024
H, DH = 16, 64
DM = H * DH  # 1024
B = 8
SCALE = DH ** (-0.5)

P = 128          # partitions
NB = 512         # moving free-dim tile
NJP = 4          # d-tile pairs (DoubleRow contracts 256 at a time)
NJ = DM // P     # 8 dout-tiles (head pairs)
HH = H // 2      # heads per kv half-tile

# fp8 scales (power-of-2, validated by numeric sim against this input
# distribution; inputs clipped to +-240 on conversion)
SX = 2.0 ** 7        # x (centered) -> fp8
SW = 2.0 ** 10       # Wkv / Wq -> fp8
KV_INV = 2.0 ** -17  # descale for kv / q PSUM (1 / (SX * SW))
SEQ = 2.0 ** 8       # eqn -> fp8 (folded into blkones = 2^-8)
SW2 = 2.0 ** 18      # W2 deviations -> fp8
Y_INV = 2.0 ** -26   # descale for y PSUM (1 / (SEQ * SW2))


def build_nc(s_len=S):
    sc = s_len // NB
    nc = bacc.Bacc(None, target_bir_lowering=False)

    x_in = nc.declare_dram_parameter("x", [sc, P, NJP, 2, NB], F8, isOutput=False)
    wq_in = nc.declare_dram_parameter("Wq", [NJP, P, 2, DM], F8, isOutput=False)
    wkv_in = nc.declare_dram_parameter("Wkv", [2, NJP, P, 2, DM], F8, isOutput=False)
    wlin_in = nc.declare_dram_parameter("Wlin", [DM, DM], F16, isOutput=False)
    qbar_in = nc.declare_dram_parameter("qbar", [NJ, P], F32, isOutput=False)
    ybias_in = nc.declare_dram_parameter("ybias", [1, DM], F32, isOutput=False)
    # tile-major output layout: [c, p, t, m] so each partition's DMA run is
    # 8KB contiguous (host inverse-permutes to [s, m] for free)
    y_out = nc.declare_dram_parameter("y", [sc, P, 4 * DM], F16, isOutput=True)

    with TileContext(nc) as tc:
        from contextlib import ExitStack

        with ExitStack() as stk:
            consts = stk.enter_context(tc.tile_pool(name="consts", bufs=1))
            wbig = stk.enter_context(tc.tile_pool(name="wbig", bufs=1))
            wqp = stk.enter_context(tc.tile_pool(name="wqp", bufs=1))
            wlp = stk.enter_context(tc.tile_pool(name="wlp", bufs=1))

            # blkones: per-head rowsum blocks scaled by 2^-8 so the
            # reciprocal yields 2^8/rowsum (folds the eqn fp8 scale)
            blkones = consts.tile([P, P], F16, tag="blkones")
            nc.vector.memset(blkones, 0.0)
            nc.vector.memset(blkones[0:64, 0:64], 1.0 / SEQ)
            nc.vector.memset(blkones[64:128, 64:128], 1.0 / SEQ)
            onescol = consts.tile([P, 1], F16, tag="onescol")
            nc.vector.memset(onescol, 1.0)
            ident32 = consts.tile([P, P], F32, tag="ident32")
            make_identity(nc, ident32)

            # ybias broadcast to all partitions via step-0 partition DMA
            ybias_bc = consts.tile([P, DM], F32, tag="ybias_bc")
            ybias_row = ybias_in[0, :]
            ybias_bcast_ap = bass.AP(
                tensor=ybias_row.tensor,
                offset=ybias_row.offset,
                ap=[[0, P]] + list(ybias_row.ap),
            )
            nc.gpsimd.dma_start(out=ybias_bc, in_=ybias_bcast_ap)

            # per-partition Exp bias columns (qbar), one col per j-group
            qbar_sb = consts.tile([P, NJ], F32, tag="qbar_sb")
            for j in range(NJ):
                nc.gpsimd.dma_start(out=qbar_sb[:, j:j + 1], in_=qbar_in[j, :])

            # blockdiag ctx staging tiles (off-diagonal zeros preset)
            bdt_tiles = []
            for j in range(NJ):
                bdt = consts.tile([P, P], F16, tag=f"bdt{j}", name=f"bdt{j}")
                nc.vector.memset(bdt, 0.0)
                bdt_tiles.append(bdt)

            # ctx [d, e] / colsum [d] accumulators (SBUF, fp32)
            ctx_acc = []
            cs_acc = []
            for j in range(NJ):
                ca = consts.tile([P, P], F32, tag=f"ctx_acc{j}")
                nc.vector.memset(ca, 0.0)
                ctx_acc.append(ca)
                cs = consts.tile([P, 1], F32, tag=f"cs_acc{j}")
                nc.vector.memset(cs, 0.0)
                cs_acc.append(cs)
            rcs2 = consts.tile([P, NJ], F32, tag="rcs2")

            # weights resident for phase A (fp8 DoubleRow pair layout;
            # wkv is h2-major so each DMA is partition-contiguous)
            wkv_sb = [
                wbig.tile([P, 2, 2, DM], F8, tag=f"w{jp}", name=f"wkv{jp}")
                for jp in range(NJP)
            ]
            wq_sb = [
                wqp.tile([P, 2, DM], F8, tag=f"q{jp}", name=f"wq{jp}")
                for jp in range(NJP)
            ]
            wlin_sb = [
                wlp.tile([P, DM], F16, tag=f"l{j}", name=f"wlin{j}")
                for j in range(NJ)
            ]

            def load_wkv():
                for h2 in range(2):
                    for jp in range(NJP):
                        nc.sync.dma_start(
                            out=wkv_sb[jp][:, h2, :, :],
                            in_=wkv_in[h2, jp],
                        )

            def load_wq():
                for jp in range(NJP):
                    nc.sync.dma_start(out=wq_sb[jp], in_=wq_in[jp])

            xt_pool = stk.enter_context(tc.tile_pool(name="xt", bufs=3))
            ek_pool = stk.enter_context(tc.tile_pool(name="ek", bufs=1))
            vt_pool = stk.enter_context(tc.tile_pool(name="vt", bufs=1))
            eq16_pool = stk.enter_context(tc.tile_pool(name="eq16", bufs=3))
            eqres_pool = stk.enter_context(tc.tile_pool(name="eqres", bufs=1))
            rr_pool = stk.enter_context(tc.tile_pool(name="rr", bufs=2))
            eq_res = [[None] * NJP for _ in range(sc)]

            w2_sb = [
                consts.tile([P, 2, DM], F8, tag=f"w2_{jp}", name=f"w2_{jp}")
                for jp in range(NJP)
            ]

            # ---------------- phase A ----------------
            with (
                tc.tile_pool(name="kvp", bufs=2, space="PSUM") as kvp_pool,
                tc.tile_pool(name="ctxp", bufs=2, space="PSUM") as ctxp_pool,
                tc.tile_pool(name="qp", bufs=2, space="PSUM") as qp_pool,
            ):
                # x chunk tiles: one 4KB-per-partition DMA per chunk on the
                # ACT HWDGE queue, prefetched two chunks ahead
                xt_bufs = [None] * sc

                def load_xt(c):
                    if c >= sc:
                        return
                    xt_b = xt_pool.tile([P, NJP, 2, NB], F8, tag="xt")
                    nc.scalar.dma_start(out=xt_b, in_=x_in[c])
                    xt_bufs[c] = xt_b

                load_xt(0)
                load_xt(1)
                for c in range(sc):
                    load_xt(c + 2)
                    xt_big = xt_bufs[c]
                    xt_tiles = [xt_big[:, jp] for jp in range(NJP)]
                    if c == 0:
                        load_wkv()
                        load_wq()
                    if c == sc - 1:
                        for j in range(NJ):
                            nc.sync.dma_start(
                                out=wlin_sb[j], in_=wlin_in[j * P:(j + 1) * P, :]
                            )

                    # kv projection (fp8 DoubleRow), two 1024-wide halves;
                    # h2-major so chunk 0 starts on the first half of Wkv
                    # while the second half is still streaming in
                    ek_tiles = [[None, None] for _ in range(4)]
                    v_tiles = [[None, None] for _ in range(4)]
                    for h2 in range(2):
                        for t in range(4):
                            kvps = kvp_pool.tile([P, DM], F32, tag="kvp")
                            for n in range(2):
                                for jp in range(NJP):
                                    nc.tensor.matmul(
                                        kvps[:, n * NB:(n + 1) * NB],
                                        xt_tiles[jp][:, :, t * P:(t + 1) * P],
                                        wkv_sb[jp][:, h2, :, n * NB:(n + 1) * NB],
                                        start=(jp == 0),
                                        stop=(jp == NJP - 1),
                                        perf_mode=DR,
                                    )
                            kv3 = kvps.rearrange("p (h c) -> p h c", h=HH)
                            ek_t = ek_pool.tile([P, HH, DH], F16, tag=f"ek{t}_{h2}")
                            nc.scalar.activation(
                                ek_t, kv3[:, :, 0:DH], AF.Exp, scale=KV_INV
                            )
                            v_t = vt_pool.tile([P, HH, DH], F16, tag=f"v{t}_{h2}")
                            nc.scalar.activation(
                                v_t, kv3[:, :, DH:2 * DH], AF.Copy, scale=KV_INV
                            )
                            ek_tiles[t][h2] = ek_t.rearrange("p h c -> p (h c)")
                            v_tiles[t][h2] = v_t.rearrange("p h c -> p (h c)")

                    # ctx^T [d, e] + colsum [d] share the ek stationary
                    for j in range(NJ):
                        h2, jl = j // 4, j % 4
                        cps = ctxp_pool.tile([P, P + 4], F32, tag="ctxp")
                        for t in range(4):
                            st = ek_tiles[t][h2][:, jl * P:(jl + 1) * P]
                            nc.tensor.matmul(
                                cps[:, 0:P],
                                st,
                                v_tiles[t][h2][:, jl * P:(jl + 1) * P],
                                start=(t == 0),
                                stop=False,
                            )
                            # start=False: ctx's t==0 start already cleared
                            # this bank's has_written bits, so the first
                            # colsum write overwrites rather than accumulates
                            nc.tensor.matmul(
                                cps[:, P:P + 1],
                                st,
                                onescol,
                                start=False,
                                stop=(t == 3),
                            )
                        nc.vector.tensor_add(ctx_acc[j], ctx_acc[j], cps[:, 0:P])
                        nc.vector.tensor_add(cs_acc[j], cs_acc[j], cps[:, P:P + 1])

                    if c == sc - 1:
                        # start the W2 finalize critical path early: colsum
                        # reciprocals (DVE) + ctx transposes (PE, fp32 mode)
                        # overlap the last chunk's q projection
                        for j in range(NJ):
                            rcs = rr_pool.tile([P, 1], F32, tag="rcs")
                            nc.vector.reciprocal_approx_fast(
                                out=rcs, in_=cs_acc[j]
                            )
                            nc.vector.tensor_scalar(
                                out=rcs2[:, j:j + 1],
                                in0=rcs,
                                scalar1=SCALE * SW2,
                                scalar2=None,
                                op0=mybir.AluOpType.mult,
                            )
                        for j in range(NJ):
                            tp = ctxp_pool.tile([P, P + 4], F32, tag="ctxp")
                            nc.tensor.transpose(
                                tp[:, 0:P], ctx_acc[j], ident32
                            )
                            nc.vector.tensor_copy(
                                bdt_tiles[j][0:64, 0:64], tp[0:64, 0:64]
                            )
                            nc.vector.tensor_copy(
                                bdt_tiles[j][64:128, 64:128], tp[64:128, 64:128]
                            )

                    # q projection (fp8 DoubleRow) -> eqn8 pairs; the
                    # rowsum matmul trails one j behind so the PE never
                    # waits on the ScalarE Exp of the current j
                    eq16s = [None] * NJ
                    def rowsum(j):
                        rsps = qp_pool.tile([P, NB], F32, tag="qp")
                        nc.tensor.matmul(rsps, blkones, eq16s[j])
                        rr = rr_pool.tile([P, NB], F32, tag="rr")
                        nc.vector.reciprocal_approx_fast(out=rr, in_=rsps)
                        nc.vector.tensor_mul(
                            eq_res[c][j // 2][:, j % 2, :], eq16s[j], rr
                        )
                    for j in range(NJ):
                        if j % 2 == 0:
                            eqp = eqres_pool.tile(
                                [P, 2, NB], F8, tag=f"eq{c}_{j // 2}"
                            )
                            eq_res[c][j // 2] = eqp
                        qps = qp_pool.tile([P, NB], F32, tag="qp")
                        for jp in range(NJP):
                            nc.tensor.matmul(
                                qps,
                                wq_sb[jp][:, :, j * P:(j + 1) * P],
                                xt_tiles[jp],
                                start=(jp == 0),
                                stop=(jp == NJP - 1),
                                perf_mode=DR,
                            )
                        if c == sc - 1:
                            # W2 finalize interleaved into the last q loop:
                            # matmuls reuse the (now idle) kv PSUM buffers;
                            # evacs alternate ScalarE / DVE so phase B isn't
                            # gated on one engine's serial chain
                            w2ps = kvp_pool.tile([P, DM], F32, tag="kvp")
                            for n in range(2):
                                nc.tensor.matmul(
                                    w2ps[:, n * NB:(n + 1) * NB],
                                    bdt_tiles[j],
                                    wlin_sb[j][:, n * NB:(n + 1) * NB],
                                )
                            dst = w2_sb[j // 2][:, j % 2, :]
                            if j % 2 == 0:
                                nc.scalar.activation(
                                    dst, w2ps, AF.Copy, scale=rcs2[:, j:j + 1]
                                )
                            else:
                                nc.vector.tensor_scalar(
                                    out=dst, in0=w2ps,
                                    scalar1=rcs2[:, j:j + 1],
                                    scalar2=None, op0=mybir.AluOpType.mult,
                                )
                        eq16 = eq16_pool.tile([P, NB], F16, tag="eq16")
                        nc.scalar.activation(
                            eq16, qps, AF.Exp,
                            scale=KV_INV, bias=qbar_sb[:, j:j + 1],
                        )
                        eq16s[j] = eq16
                        if j > 0:
                            rowsum(j - 1)
                    rowsum(NJ - 1)

            y_pool = stk.enter_context(tc.tile_pool(name="ysb", bufs=3))
            yd_pool = stk.enter_context(tc.tile_pool(name="yd", bufs=3))

            # ---------------- phase B: y = eqn8 @ W2d (fp8 DoubleRow) ---
            with tc.tile_pool(name="yp", bufs=3, space="PSUM") as yp_pool:
                for c in range(sc):
                    ysb4 = y_pool.tile([P, 4, DM], F16, tag="ysb")
                    for t in range(4):
                        yps = yp_pool.tile([P, DM], F32, tag="yp")
                        for n in range(2):
                            for jp in range(NJP):
                                nc.tensor.matmul(
                                    yps[:, n * NB:(n + 1) * NB],
                                    eq_res[c][jp][:, :, t * P:(t + 1) * P],
                                    w2_sb[jp][:, :, n * NB:(n + 1) * NB],
                                    start=(jp == 0),
                                    stop=(jp == NJP - 1),
                                    perf_mode=DR,
                                )
                        yd = yd_pool.tile([P, DM], F32, tag="yd")
                        nc.scalar.activation(yd, yps, AF.Copy, scale=Y_INV)
                        nc.vector.tensor_add(ysb4[:, t, :], yd, ybias_bc)
                        if c == sc - 1:
                            # last chunk: per-tile DMAs so the final
                            # evac->DMA chain drains in small pieces
                            nc.sync.dma_start(
                                out=y_out[c, :, t * DM:(t + 1) * DM],
                                in_=ysb4[:, t, :],
                            )
                    if c < sc - 1:
                        # one batched 8KB-per-partition DMA per chunk
                        nc.sync.dma_start(out=y_out[c], in_=ysb4)
    nc.compile()
    return nc


def _q8(a, scale):
    return np.clip(
        np.asarray(a, dtype=np.float32) * scale, -240.0, 240.0
    ).astype(ml_dtypes.float8_e4m3)


def prepare_inputs(x, Wq, Wkv, Wlin, blin):
    """Host-side packing: returns per-core input maps."""
    x = np.asarray(x, dtype=np.float32)
    s_len = x.shape[1]
    sc = s_len // NB
    Wq64 = np.asarray(Wq, dtype=np.float64)
    Wkv64 = np.asarray(Wkv, dtype=np.float64)
    Wlin64 = np.asarray(Wlin, dtype=np.float64)
    blin64 = np.asarray(blin, dtype=np.float64).reshape(DM)

    # DoubleRow pair layouts (logical contraction row d = jp*256 + ko*128 + p)
    # wkv: [h2, jp, p, ko, n] — partition-contiguous per (h2, jp) tile
    wkv8 = _q8(
        np.asarray(Wkv, np.float32).reshape(NJP, 2, P, 2, DM)
        .transpose(3, 0, 2, 1, 4), SW,
    )
    wq8 = _q8(
        np.asarray(Wq, np.float32).reshape(NJP, 2, P, DM)
        .transpose(0, 2, 1, 3), SW,
    )
    wlin16 = np.asarray(Wlin, np.float32).astype(np.float16)

    in_maps = []
    for b in range(x.shape[0]):
        xb = x[b].astype(np.float64)
        xm = xb.mean(axis=0)                      # [D]
        xc = (xb - xm).astype(np.float32)
        # x: [c, p, jp, ko, s'] — one 4KB-per-partition tile per chunk
        xdr = _q8(
            np.ascontiguousarray(
                xc.T.reshape(NJP, 2, P, sc, NB).transpose(3, 2, 0, 1, 4)
            ), SX,
        )
        qbar = (xm @ Wq64).astype(np.float32).reshape(NJ, P)
        vbar = (xm @ Wkv64).reshape(H, 2 * DH)[:, DH:]        # [H, DH]
        w2bar_sum = SCALE * (vbar.reshape(DM) @ Wlin64)       # [DM]
        ybias = (w2bar_sum + blin64).astype(np.float32).reshape(1, DM)
        in_maps.append({
            "x": np.ascontiguousarray(xdr),
            "Wq": np.ascontiguousarray(wq8),
            "Wkv": np.ascontiguousarray(wkv8),
            "Wlin": wlin16,
            "qbar": np.ascontiguousarray(qbar),
            "ybias": ybias,
        })
    return in_maps


def kernel(x, Wq, Wkv, Wlin, blin):
    from concourse.bass_utils import run_bass_kernel_spmd

    x = np.asarray(x, dtype=np.float32)
    b = x.shape[0]
    nc = build_nc(x.shape[1])
    in_maps = prepare_inputs(x, Wq, Wkv, Wlin, blin)
    res = run_bass_kernel_spmd(nc, in_maps, list(range(b)))
    sc = x.shape[1] // NB
    return np.stack([
        res.results[i]["y"].astype(np.float32)
        .reshape(sc, P, 4, DM).transpose(0, 2, 1, 3).reshape(x.shape[1], DM)
        for i in range(b)
    ])


if __name__ == "__main__":
    rng = np.random.default_rng(0)
    x = rng.random((B, S, D), dtype=np.float32)
    Wq = (rng.standard_normal((D, DM)) * 0.02).astype(np.float32)
    Wkv = (rng.standard_normal((D, 2 * DM)) * 0.02).astype(np.float32)
    Wlin = (rng.standard_normal((DM, DM)) * 0.02).astype(np.float32)
    blin = np.zeros((DM,), dtype=np.float32)
    y = kernel(x=x, Wq=Wq, Wkv=Wkv, Wlin=Wlin, blin=blin)
    print(y.shape, y.dtype)
